# revision 28
# baseline (speedup 1.0000x reference)
"""GQA attention (RoPE + causal softmax + o_proj) on 8 Trainium2 NeuronCores.

Sharding: core = b*4 + g where b = batch (2), g = head-group (4).
Each core handles 8 query heads (global 8g..8g+7) and their 2 KV heads
(2g, 2g+1) for one batch element, producing a partial o_proj output
(contraction over its 512 of the 2048 hd dims). The host sums the 4
partials per batch element.

Per-core data layout (all matmul operands bf16, fp32 PSUM accumulation):
  - x arrives pre-transposed (hid, S) so QKV projections contract hid on
    partitions.
  - q^T/k^T are built per 128-row chunk pairing heads (i, i+4) so each
    head's 64 rows sit at the same partition base (0 or 64) as its KV
    head, enabling concurrent 64-row-group score matmuls.
  - Scores are computed transposed: S^T[k, q] = k^T.T @ q^T, softmax
    denominator via a ones column appended to v in the AV matmul
    (PSUM row 64 = sum over k of exp scores).
  - exp (with the 1/sqrt(D) scale folded in) runs on ScalarE straight
    out of PSUM; causal masking multiplies a 128x128 triangular tile
    only on diagonal blocks; fully-masked regions are never computed.
"""

import numpy as np
import ml_dtypes
from contextlib import ExitStack

import concourse.mybir as mybir
from concourse import bacc
from concourse.tile import TileContext
from concourse.bass_utils import run_bass_kernel_spmd

BF16 = mybir.dt.bfloat16
F32 = mybir.dt.float32
NP_BF16 = ml_dtypes.bfloat16

HID = 2048
D = 64
H = 32           # global query heads
KV = 8           # global kv heads
B = 2
P = 128
SC = 512         # q-chunk width (also matmul free dim / PSUM bank)

_CACHE = {}


def build_nc(S):
    assert S % SC == 0
    NHID = HID // P       # hid chunks (16)
    NSB = S // P          # 128-row s-blocks
    NSC = S // SC         # 512-col s-chunks
    QCH = 4               # q chunk-pairs
    EXP = mybir.ActivationFunctionType.Exp

    nc = bacc.Bacc("TRN2", target_bir_lowering=False, debug=False)
    xT = nc.dram_tensor("xT", [HID, S], BF16, kind="ExternalInput")
    wqkv = nc.dram_tensor("wqkv", [HID, 640], BF16, kind="ExternalInput")
    wv = nc.dram_tensor("wv", [HID, 128], BF16, kind="ExternalInput")
    wo = nc.dram_tensor("wo", [512, HID], BF16, kind="ExternalInput")
    cosT = nc.dram_tensor("cosT", [128, S], BF16, kind="ExternalInput")
    sinT = nc.dram_tensor("sinT", [128, S], BF16, kind="ExternalInput")
    trimask = nc.dram_tensor("trimask", [128, 128], BF16, kind="ExternalInput")
    o_part = nc.dram_tensor("o_part", [S, HID], F32, kind="ExternalOutput")

    with TileContext(nc) as tc, ExitStack() as ctx:
        res = ctx.enter_context(tc.tile_pool(name="res", bufs=1))
        rope = ctx.enter_context(tc.tile_pool(name="rope", bufs=2))
        ptp = ctx.enter_context(tc.tile_pool(name="ptp", bufs=8))
        rbp = ctx.enter_context(tc.tile_pool(name="rbp", bufs=2))
        obp = ctx.enter_context(tc.tile_pool(name="obp", bufs=3))
        psum = ctx.enter_context(tc.tile_pool(name="psum", bufs=1, space="PSUM"))

        # resident loads
        cos_sb = res.tile([P, S], BF16, tag="cos")
        nc.sync.dma_start(out=cos_sb, in_=cosT[:, :])
        sin_sb = res.tile([P, S], BF16, tag="sin")
        nc.sync.dma_start(out=sin_sb, in_=sinT[:, :])
        mask_sb = res.tile([P, P], BF16, tag="mask")
        nc.sync.dma_start(out=mask_sb, in_=trimask[:, :])
        # load order matches first-consumption order (proj loop walks h)
        xt_sb, wqkv_sb, wv_sb = [], [], []
        for h in range(NHID):
            t = res.tile([P, S], BF16, tag=f"xt{h}", name=f"xt{h}")
            nc.sync.dma_start(out=t, in_=xT[h * P:(h + 1) * P, :])
            xt_sb.append(t)
            t = res.tile([P, 640], BF16, tag=f"wqkv{h}", name=f"wqkv{h}")
            nc.sync.dma_start(out=t, in_=wqkv[h * P:(h + 1) * P, :])
            wqkv_sb.append(t)
            t = res.tile([P, 128], BF16, tag=f"wv{h}", name=f"wv{h}")
            nc.sync.dma_start(out=t, in_=wv[h * P:(h + 1) * P, :])
            wv_sb.append(t)
        wo_sb = []
        for i in range(4):
            t = res.tile([P, HID], BF16, tag=f"wo{i}", name=f"wo{i}")
            nc.sync.dma_start(out=t, in_=wo[i * P:(i + 1) * P, :])
            wo_sb.append(t)

        # ---- Phase A: q^T/k^T projection + RoPE ----
        # chunks 0-3: q head pairs (i, i+4); chunk 4: k (kv0 rows 0-63, kv1 rows 64-127)
        qkrot = []
        for m in range(5):
            t = res.tile([P, S], BF16, tag=f"qkrot{m}", name=f"qkrot{m}")
            qkrot.append(t)
        # v tiles [128, 194]: [v0(0:64) | 1 | 0 | v1(66:130) | 1 | 0-pad] —
        # both AV stationaries are 128 columns (FWL) at 4B-aligned offsets
        vnat = [res.tile([P, 194], BF16, tag=f"vnat{sb}", name=f"vnat{sb}")
                for sb in range(NSB)]
        attnT = []
        for i in range(QCH):
            t = res.tile([P, S], BF16, tag=f"attnT{i}", name=f"attnT{i}")
            attnT.append(t)

        def emit_o_unit(qb, n):
            po = psum.tile([P, SC], F32, tag="acc", bufs=4, name="po")
            for i in range(QCH):
                nc.tensor.matmul(
                    po,
                    lhsT=attnT[i][:, qb * P:(qb + 1) * P],
                    rhs=wo_sb[i][:, n * SC:(n + 1) * SC],
                    start=(i == 0),
                    stop=(i == QCH - 1),
                )
            ob = obp.tile([P, SC], F32, tag="ob", name="ob")
            nc.vector.tensor_copy(ob, po)
            nc.sync.dma_start(
                out=o_part[qb * P:(qb + 1) * P, n * SC:(n + 1) * SC], in_=ob
            )

        def gen_proj_schunk(s):
            """Emit s-chunk s projections + RoPE + v as ~1us units (yields)."""
            sl = slice(s * SC, (s + 1) * SC)
            for m in (4, 0, 1, 2, 3):
                ps = psum.tile([P, SC], F32, tag="acc", bufs=4, name="ps_proj")
                for h0 in range(0, NHID, 4):
                    for h in range(h0, h0 + 4):
                        nc.tensor.matmul(
                            ps,
                            lhsT=wqkv_sb[h][:, m * P:(m + 1) * P],
                            rhs=xt_sb[h][:, sl],
                            start=(h == 0),
                            stop=(h == NHID - 1),
                        )
                    yield
                # rotate_half operand: engines are lane-locked, so the
                # +-32-partition swap must go through DMA (SBUF->SBUF)
                qraw = rope.tile([P, SC], BF16, tag="qraw", bufs=2, name="qraw")
                nc.scalar.copy(qraw, ps)
                qswp = rope.tile([P, SC], BF16, tag="qswp", bufs=2, name="qswp")
                for dst, src in ((0, 32), (32, 0), (64, 96), (96, 64)):
                    nc.sync.dma_start(
                        out=qswp[dst:dst + 32, :], in_=qraw[src:src + 32, :]
                    )
                t1 = rope.tile([P, SC], BF16, tag="t1", bufs=2, name="t1")
                nc.vector.tensor_mul(t1, ps, cos_sb[:, sl])
                t2 = rope.tile([P, SC], BF16, tag="t2", bufs=2, name="t2")
                nc.gpsimd.tensor_mul(t2, qswp, sin_sb[:, sl])
                nc.vector.tensor_add(qkrot[m][:, sl], t1, t2)
                yield
            for sb in range(4 * s, 4 * s + 4):
                t = vnat[sb]
                nc.gpsimd.memset(t[:, 64:65], 1.0)
                nc.gpsimd.memset(t[:, 65:66], 0.0)
                nc.gpsimd.memset(t[:, 130:131], 1.0)
                nc.gpsimd.memset(t[:, 131:194], 0.0)
                pv = psum.tile([P, 128], F32, tag="acc", bufs=4, name="ps_v")
                for h0 in range(0, NHID, 8):
                    for h in range(h0, h0 + 8):
                        nc.tensor.matmul(
                            pv,
                            lhsT=xt_sb[h][:, sb * P:(sb + 1) * P],
                            rhs=wv_sb[h],
                            start=(h == 0),
                            stop=(h == NHID - 1),
                        )
                    yield
                nc.vector.tensor_copy(t[:, 0:64], pv[:, 0:64])
                nc.vector.tensor_copy(t[:, 66:130], pv[:, 64:128])
                yield

        def gen_o_chunk(c):
            for qb in range(4 * c, 4 * c + 4):
                for n in range(HID // SC):
                    emit_o_unit(qb, n)
                    yield

        def interleave(*gens):
            """Round-robin the generators, one unit per yield."""
            gens = [g for g in gens if g is not None]
            i = 0
            while gens:
                g = gens[i % len(gens)]
                try:
                    next(g)
                except StopIteration:
                    gens.remove(g)
                    continue
                yield
                i += 1

        # ---- s-chunk 0 projections emitted dense (nothing to overlap) ----
        with nc.named_scope("projA0"):
            for _ in gen_proj_schunk(0):
                pass

        # ---- attention chunks; the next s-chunk's projections and the
        # previous chunk's o_proj drip into the kb loop as PE fill work
        # while ScalarE streams exps (the per-kb rate limiter) ----
        for c in range(NSC):
          with nc.named_scope(f"attn_c{c}"):
            q0 = c * SC
            nkb = 4 * c + 4
            filler = interleave(
                gen_proj_schunk(c + 1) if c + 1 < NSC else None,
                gen_o_chunk(c - 1) if c >= 1 else None,
            )
            n_units = (37 if c + 1 < NSC else 0) + (16 if c >= 1 else 0)
            total_iters = 4 * nkb
            it = 0
            spent = 0
            for pg in (0, 1, 2, 3):
                avs = {}
                for hp in (pg,):
                    a0 = psum.tile([P, SC], F32, tag="acc", bufs=4, name="av0")
                    a1 = psum.tile([P, SC], F32, tag="acc", bufs=4, name="av1")
                    avs[hp] = (a0, a1)
                for kb in range(nkb):
                    vs = max(0, (kb - 4 * c) * P)  # first valid col in chunk
                    it += 1
                    want = (it * n_units) // total_iters
                    while spent < want:
                        try:
                            next(filler)
                            spent += 1
                        except StopIteration:
                            spent = want
                            break
                    for hp in (pg,):
                        av0, av1 = avs[hp]
                        st = psum.tile([P, 2 * SC], F32, tag="st", bufs=2, name="st")
                        nc.tensor.matmul(
                            st[:, vs:SC],
                            lhsT=qkrot[4][0:64, kb * P:(kb + 1) * P],
                            rhs=qkrot[hp][0:64, q0 + vs:q0 + SC],
                            start=True, stop=True,
                        )
                        nc.tensor.matmul(
                            st[:, SC:2 * SC],
                            lhsT=qkrot[4][64:128, kb * P:(kb + 1) * P],
                            rhs=qkrot[hp][64:128, q0:q0 + SC],
                            start=True, stop=True,
                        )
                        pt = ptp.tile([P, 2 * SC], BF16, tag="pt", name="pt")
                        # one exp over [vs:1024]: the dead span [SC:SC+vs]
                        # is exp'd too (never read) to save an instruction
                        nc.scalar.activation(
                            pt[:, vs:2 * SC], st[:, vs:2 * SC], EXP, scale=0.125
                        )
                        if kb - 4 * c >= 0:  # diagonal block: mask triangle
                            nc.vector.tensor_mul(
                                pt[:, vs:vs + P], pt[:, vs:vs + P], mask_sb
                            )
                            nc.vector.tensor_mul(
                                pt[:, SC + vs:SC + vs + P],
                                pt[:, SC + vs:SC + vs + P], mask_sb
                            )
                        nc.tensor.matmul(
                            av0[:, vs:SC],
                            lhsT=vnat[kb][:, 0:128],
                            rhs=pt[:, vs:SC],
                            start=(kb == 0), stop=(kb == nkb - 1),
                        )
                        nc.tensor.matmul(
                            av1[:, vs:SC],
                            lhsT=vnat[kb][:, 66:194],
                            rhs=pt[:, SC + vs:2 * SC],
                            start=(kb == 0), stop=(kb == nkb - 1),
                        )
                for hp in (pg,):
                    # normalize: attnT rows = 0 (head hp) / 64 (head hp+4)
                    for av, rbase in ((avs[hp][0], 0), (avs[hp][1], 64)):
                        # compute stays partition-aligned; cross-partition
                        # moves (row 64 -> 0, result -> rows 64..127) via DMA.
                        # reciprocal_approx_fast needs base partition 0.
                        den = rbp.tile([65, SC], F32, tag="den", bufs=2, name="den")
                        nc.vector.tensor_copy(den[64:65, :], av[64:65, :])
                        den0 = rbp.tile([1, SC], F32, tag="den0", bufs=2, name="den0")
                        nc.sync.dma_start(out=den0, in_=den[64:65, :])
                        rec0 = rbp.tile([1, SC], F32, tag="rec0", bufs=2, name="rec0")
                        nc.vector.reciprocal_approx_fast(rec0, den0)
                        rb = rbp.tile([64, SC], F32, tag="rb", bufs=2, name="rb")
                        nc.gpsimd.partition_broadcast(rb, rec0)
                        if rbase == 0:
                            nc.vector.tensor_mul(
                                attnT[hp][0:64, q0:q0 + SC], av[0:64, :], rb
                            )
                        else:
                            hi = rbp.tile([64, SC], BF16, tag="hi", bufs=2, name="hi")
                            nc.vector.tensor_mul(hi, av[0:64, :], rb)
                            nc.sync.dma_start(
                                out=attnT[hp][64:128, q0:q0 + SC], in_=hi
                            )

            # drain remaining filler (next chunk depends on its qkrot/vnat)
            for _ in filler:
                pass
        # last chunk's o_proj tail
        for _ in gen_o_chunk(NSC - 1):
            pass

    nc.finalize()
    return nc


def prep_core_inputs(x, cos, sin, wq, wk, wv, wo, core, _shared={}):
    """Build the per-core input map (all host-side numpy)."""
    b, g = core // 4, core % 4
    S = x.shape[1]

    key = ("xT", b, id(x))
    if key not in _shared:
        _shared.clear() if len(_shared) > 8 else None
        _shared[key] = np.ascontiguousarray(x[b].T).astype(NP_BF16)
    xT = _shared[key]

    heads = [8 * g + i for i in range(4) for _ in (0,)]  # chunk bases
    qcols = []
    for i in range(4):
        h0, h1 = 8 * g + i, 8 * g + i + 4
        qcols.append(wq[:, h0 * D:(h0 + 1) * D])
        qcols.append(wq[:, h1 * D:(h1 + 1) * D])
    kcols = wk[:, 2 * g * D:(2 * g + 2) * D]
    wqkv_c = np.concatenate(qcols + [kcols], axis=1).astype(NP_BF16)
    wv_c = np.ascontiguousarray(wv[:, 2 * g * D:(2 * g + 2) * D]).astype(NP_BF16)
    worows = []
    for i in range(4):
        h0, h1 = 8 * g + i, 8 * g + i + 4
        worows.append(wo[h0 * D:(h0 + 1) * D, :])
        worows.append(wo[h1 * D:(h1 + 1) * D, :])
    wo_c = np.concatenate(worows, axis=0).astype(NP_BF16)

    cosT = np.tile(cos[:S].T, (2, 1)).astype(NP_BF16)
    sinT_h = np.concatenate([-sin[:S].T[:D // 2], sin[:S].T[D // 2:]], axis=0)
    sinT = np.tile(sinT_h, (2, 1)).astype(NP_BF16)
    trimask = np.triu(np.ones((P, P), dtype=NP_BF16))

    return {
        "xT": xT, "wqkv": wqkv_c, "wv": wv_c, "wo": wo_c,
        "cosT": cosT, "sinT": sinT, "trimask": trimask,
    }


def kernel(x, cos, sin, wq, wk, wv, wo):
    x = np.asarray(x)
    S = x.shape[1]
    assert x.shape == (B, S, HID)
    if S not in _CACHE:
        _CACHE[S] = build_nc(S)
    nc = _CACHE[S]
    in_maps = [
        prep_core_inputs(x, np.asarray(cos), np.asarray(sin), np.asarray(wq),
                         np.asarray(wk), np.asarray(wv), np.asarray(wo), core)
        for core in range(8)
    ]
    res = run_bass_kernel_spmd(nc, in_maps, core_ids=list(range(8)))
    out = np.zeros((B, S, HID), np.float32)
    for core in range(8):
        out[core // 4] += res.results[core]["o_part"]
    return out


# revision 30
# speedup vs baseline: 1.1076x; 1.1076x over previous
"""GQA attention (RoPE + causal softmax + o_proj) on 8 Trainium2 NeuronCores.

Sharding: core = b*4 + g where b = batch (2), g = head-group (4).
Each core handles 8 query heads (global 8g..8g+7) and their 2 KV heads
(2g, 2g+1) for one batch element, producing a partial o_proj output
(contraction over its 512 of the 2048 hd dims). The host sums the 4
partials per batch element.

Per-core data layout (all matmul operands bf16, fp32 PSUM accumulation):
  - x arrives pre-transposed (hid, S) so QKV projections contract hid on
    partitions.
  - q^T/k^T are built per 128-row chunk pairing heads (i, i+4) so each
    head's 64 rows sit at the same partition base (0 or 64) as its KV
    head, enabling concurrent 64-row-group score matmuls.
  - Scores are computed transposed: S^T[k, q] = k^T.T @ q^T, softmax
    denominator via a ones column appended to v in the AV matmul
    (PSUM row 64 = sum over k of exp scores).
  - exp (with the 1/sqrt(D) scale folded in) runs on ScalarE straight
    out of PSUM; causal masking multiplies a 128x128 triangular tile
    only on diagonal blocks; fully-masked regions are never computed.
"""

import numpy as np
import ml_dtypes
from contextlib import ExitStack

import concourse.mybir as mybir
from concourse import bacc
from concourse.tile import TileContext
from concourse.bass_utils import run_bass_kernel_spmd

BF16 = mybir.dt.bfloat16
F32 = mybir.dt.float32
NP_BF16 = ml_dtypes.bfloat16

HID = 2048
D = 64
H = 32           # global query heads
KV = 8           # global kv heads
B = 2
P = 128
SC = 512         # q-chunk width (also matmul free dim / PSUM bank)

_CACHE = {}


def build_nc(S):
    assert S % SC == 0
    NHID = HID // P       # hid chunks (16)
    NSB = S // P          # 128-row s-blocks
    NSC = S // SC         # 512-col s-chunks
    QCH = 4               # q chunk-pairs
    EXP = mybir.ActivationFunctionType.Exp

    nc = bacc.Bacc("TRN2", target_bir_lowering=False, debug=False)
    xT = nc.dram_tensor("xT", [HID, S], BF16, kind="ExternalInput")
    wqkv = nc.dram_tensor("wqkv", [HID, 640], BF16, kind="ExternalInput")
    wv = nc.dram_tensor("wv", [HID, 128], BF16, kind="ExternalInput")
    wo = nc.dram_tensor("wo", [512, HID], BF16, kind="ExternalInput")
    cosT = nc.dram_tensor("cosT", [128, S], BF16, kind="ExternalInput")
    sinT = nc.dram_tensor("sinT", [128, S], BF16, kind="ExternalInput")
    trimask = nc.dram_tensor("trimask", [128, 128], BF16, kind="ExternalInput")
    o_part = nc.dram_tensor("o_part", [S, HID], F32, kind="ExternalOutput")

    with TileContext(nc) as tc, ExitStack() as ctx:
        res = ctx.enter_context(tc.tile_pool(name="res", bufs=1))
        rope = ctx.enter_context(tc.tile_pool(name="rope", bufs=2))
        ptp = ctx.enter_context(tc.tile_pool(name="ptp", bufs=8))
        rbp = ctx.enter_context(tc.tile_pool(name="rbp", bufs=2))
        obp = ctx.enter_context(tc.tile_pool(name="obp", bufs=3))
        psum = ctx.enter_context(tc.tile_pool(name="psum", bufs=1, space="PSUM"))

        # resident loads
        cos_sb = res.tile([P, S], BF16, tag="cos")
        nc.sync.dma_start(out=cos_sb, in_=cosT[:, :])
        sin_sb = res.tile([P, S], BF16, tag="sin")
        nc.sync.dma_start(out=sin_sb, in_=sinT[:, :])
        mask_sb = res.tile([P, P], BF16, tag="mask")
        nc.sync.dma_start(out=mask_sb, in_=trimask[:, :])
        # load order matches first-consumption order (proj loop walks h)
        xt_sb, wqkv_sb, wv_sb = [], [], []
        for h in range(NHID):
            t = res.tile([P, S], BF16, tag=f"xt{h}", name=f"xt{h}")
            nc.sync.dma_start(out=t, in_=xT[h * P:(h + 1) * P, :])
            xt_sb.append(t)
            t = res.tile([P, 640], BF16, tag=f"wqkv{h}", name=f"wqkv{h}")
            nc.sync.dma_start(out=t, in_=wqkv[h * P:(h + 1) * P, :])
            wqkv_sb.append(t)
            t = res.tile([P, 128], BF16, tag=f"wv{h}", name=f"wv{h}")
            nc.sync.dma_start(out=t, in_=wv[h * P:(h + 1) * P, :])
            wv_sb.append(t)
        wo_sb = []
        for i in range(4):
            t = res.tile([P, HID], BF16, tag=f"wo{i}", name=f"wo{i}")
            nc.sync.dma_start(out=t, in_=wo[i * P:(i + 1) * P, :])
            wo_sb.append(t)

        # ---- Phase A: q^T/k^T projection + RoPE ----
        # chunks 0-3: q head pairs (i, i+4); chunk 4: k (kv0 rows 0-63, kv1 rows 64-127)
        qkrot = []
        for m in range(5):
            t = res.tile([P, S], BF16, tag=f"qkrot{m}", name=f"qkrot{m}")
            qkrot.append(t)
        # v tiles [128, 194]: [v0(0:64) | 1 | 0 | v1(66:130) | 1 | 0-pad] —
        # both AV stationaries are 128 columns (FWL) at 4B-aligned offsets
        vnat = [res.tile([P, 194], BF16, tag=f"vnat{sb}", name=f"vnat{sb}")
                for sb in range(NSB)]
        attnT = []
        for i in range(QCH):
            t = res.tile([P, S], BF16, tag=f"attnT{i}", name=f"attnT{i}")
            attnT.append(t)

        def emit_o_unit(qb, n):
            po = psum.tile([P, SC], F32, tag="acc", bufs=4, name="po")
            for i in range(QCH):
                nc.tensor.matmul(
                    po,
                    lhsT=attnT[i][:, qb * P:(qb + 1) * P],
                    rhs=wo_sb[i][:, n * SC:(n + 1) * SC],
                    start=(i == 0),
                    stop=(i == QCH - 1),
                )
            ob = obp.tile([P, SC], F32, tag="ob", name="ob")
            nc.vector.tensor_copy(ob, po)
            nc.sync.dma_start(
                out=o_part[qb * P:(qb + 1) * P, n * SC:(n + 1) * SC], in_=ob
            )

        def gen_proj_schunk(s):
            """Emit s-chunk s projections + RoPE + v as units (yields).

            Matmul groups stay consecutive (no spread) and eviction units
            only follow completed groups, so cross-engine ops never
            head-of-line-block the in-order ACT/DVE streams."""
            sl = slice(s * SC, (s + 1) * SC)
            for m in (4, 0, 1, 2, 3):
                ps = psum.tile([P, SC], F32, tag="acc", bufs=4, name="ps_proj")
                for h0 in (0, 8):
                    for h in range(h0, h0 + 8):
                        nc.tensor.matmul(
                            ps,
                            lhsT=wqkv_sb[h][:, m * P:(m + 1) * P],
                            rhs=xt_sb[h][:, sl],
                            start=(h == 0),
                            stop=(h == NHID - 1),
                        )
                    yield
                # rotate_half operand: engines are lane-locked, so the
                # +-32-partition swap must go through DMA (SBUF->SBUF)
                qraw = rope.tile([P, SC], BF16, tag="qraw", bufs=2, name="qraw")
                nc.scalar.copy(qraw, ps)
                qswp = rope.tile([P, SC], BF16, tag="qswp", bufs=2, name="qswp")
                for dst, src in ((0, 32), (32, 0), (64, 96), (96, 64)):
                    nc.sync.dma_start(
                        out=qswp[dst:dst + 32, :], in_=qraw[src:src + 32, :]
                    )
                t1 = rope.tile([P, SC], BF16, tag="t1", bufs=2, name="t1")
                nc.vector.tensor_mul(t1, ps, cos_sb[:, sl])
                t2 = rope.tile([P, SC], BF16, tag="t2", bufs=2, name="t2")
                nc.gpsimd.tensor_mul(t2, qswp, sin_sb[:, sl])
                nc.vector.tensor_add(qkrot[m][:, sl], t1, t2)
                yield
            for sb in range(4 * s, 4 * s + 4):
                t = vnat[sb]
                nc.gpsimd.memset(t[:, 64:65], 1.0)
                nc.gpsimd.memset(t[:, 65:66], 0.0)
                nc.gpsimd.memset(t[:, 130:131], 1.0)
                nc.gpsimd.memset(t[:, 131:194], 0.0)
                pv = psum.tile([P, 128], F32, tag="acc", bufs=4, name="ps_v")
                for h in range(NHID):
                    nc.tensor.matmul(
                        pv,
                        lhsT=xt_sb[h][:, sb * P:(sb + 1) * P],
                        rhs=wv_sb[h],
                        start=(h == 0),
                        stop=(h == NHID - 1),
                    )
                yield
                nc.vector.tensor_copy(t[:, 0:64], pv[:, 0:64])
                nc.vector.tensor_copy(t[:, 66:130], pv[:, 64:128])
                yield

        def gen_o_chunk(c):
            for qb in range(4 * c, 4 * c + 4):
                for n in range(HID // SC):
                    emit_o_unit(qb, n)
                    yield

        def interleave(*gens):
            """Round-robin the generators, one unit per yield."""
            gens = [g for g in gens if g is not None]
            i = 0
            while gens:
                g = gens[i % len(gens)]
                try:
                    next(g)
                except StopIteration:
                    gens.remove(g)
                    continue
                yield
                i += 1

        # ---- s-chunk 0 projections emitted dense (nothing to overlap) ----
        with nc.named_scope("projA0"):
            for _ in gen_proj_schunk(0):
                pass

        # ---- attention chunks; the next s-chunk's projections and the
        # previous chunk's o_proj drip into the kb loop as PE fill work
        # while ScalarE streams exps (the per-kb rate limiter) ----
        for c in range(NSC):
          with nc.named_scope(f"attn_c{c}"):
            q0 = c * SC
            nkb = 4 * c + 4
            filler = interleave(
                gen_proj_schunk(c + 1) if c + 1 < NSC else None,
                gen_o_chunk(c - 1) if c >= 1 else None,
            )
            n_units = (23 if c + 1 < NSC else 0) + (16 if c >= 1 else 0)
            total_iters = 4 * nkb
            it = 0
            spent = 0
            for pg in (0, 1, 2, 3):
                avs = {}
                for hp in (pg,):
                    a0 = psum.tile([P, SC], F32, tag="acc", bufs=4, name="av0")
                    a1 = psum.tile([P, SC], F32, tag="acc", bufs=4, name="av1")
                    avs[hp] = (a0, a1)
                for kb in range(nkb):
                    vs = max(0, (kb - 4 * c) * P)  # first valid col in chunk
                    for hp in (pg,):
                        av0, av1 = avs[hp]
                        st = psum.tile([P, 2 * SC], F32, tag="st", bufs=2, name="st")
                        nc.tensor.matmul(
                            st[:, vs:SC],
                            lhsT=qkrot[4][0:64, kb * P:(kb + 1) * P],
                            rhs=qkrot[hp][0:64, q0 + vs:q0 + SC],
                            start=True, stop=True,
                        )
                        nc.tensor.matmul(
                            st[:, SC:2 * SC],
                            lhsT=qkrot[4][64:128, kb * P:(kb + 1) * P],
                            rhs=qkrot[hp][64:128, q0:q0 + SC],
                            start=True, stop=True,
                        )
                        pt = ptp.tile([P, 2 * SC], BF16, tag="pt", name="pt")
                        # one exp over [vs:1024]: the dead span [SC:SC+vs]
                        # is exp'd too (never read) to save an instruction
                        nc.scalar.activation(
                            pt[:, vs:2 * SC], st[:, vs:2 * SC], EXP, scale=0.125
                        )
                        if kb - 4 * c >= 0:  # diagonal block: mask triangle
                            nc.vector.tensor_mul(
                                pt[:, vs:vs + P], pt[:, vs:vs + P], mask_sb
                            )
                            nc.vector.tensor_mul(
                                pt[:, SC + vs:SC + vs + P],
                                pt[:, SC + vs:SC + vs + P], mask_sb
                            )
                        nc.tensor.matmul(
                            av0[:, vs:SC],
                            lhsT=vnat[kb][:, 0:128],
                            rhs=pt[:, vs:SC],
                            start=(kb == 0), stop=(kb == nkb - 1),
                        )
                        nc.tensor.matmul(
                            av1[:, vs:SC],
                            lhsT=vnat[kb][:, 66:194],
                            rhs=pt[:, SC + vs:2 * SC],
                            start=(kb == 0), stop=(kb == nkb - 1),
                        )
                    it += 1
                    want = (it * n_units) // total_iters
                    while spent < want:
                        try:
                            next(filler)
                            spent += 1
                        except StopIteration:
                            spent = want
                            break
                for hp in (pg,):
                    # normalize: attnT rows = 0 (head hp) / 64 (head hp+4)
                    for av, rbase in ((avs[hp][0], 0), (avs[hp][1], 64)):
                        # compute stays partition-aligned; cross-partition
                        # moves (row 64 -> 0, result -> rows 64..127) via DMA.
                        # reciprocal_approx_fast needs base partition 0.
                        den = rbp.tile([65, SC], F32, tag="den", bufs=2, name="den")
                        nc.vector.tensor_copy(den[64:65, :], av[64:65, :])
                        den0 = rbp.tile([1, SC], F32, tag="den0", bufs=2, name="den0")
                        nc.sync.dma_start(out=den0, in_=den[64:65, :])
                        rec0 = rbp.tile([1, SC], F32, tag="rec0", bufs=2, name="rec0")
                        nc.vector.reciprocal_approx_fast(rec0, den0)
                        rb = rbp.tile([64, SC], F32, tag="rb", bufs=2, name="rb")
                        nc.gpsimd.partition_broadcast(rb, rec0)
                        if rbase == 0:
                            nc.vector.tensor_mul(
                                attnT[hp][0:64, q0:q0 + SC], av[0:64, :], rb
                            )
                        else:
                            hi = rbp.tile([64, SC], BF16, tag="hi", bufs=2, name="hi")
                            nc.vector.tensor_mul(hi, av[0:64, :], rb)
                            nc.sync.dma_start(
                                out=attnT[hp][64:128, q0:q0 + SC], in_=hi
                            )

            # drain remaining filler (next chunk depends on its qkrot/vnat)
            for _ in filler:
                pass
        # last chunk's o_proj tail
        for _ in gen_o_chunk(NSC - 1):
            pass

    nc.finalize()
    return nc


def prep_core_inputs(x, cos, sin, wq, wk, wv, wo, core, _shared={}):
    """Build the per-core input map (all host-side numpy)."""
    b, g = core // 4, core % 4
    S = x.shape[1]

    key = ("xT", b, id(x))
    if key not in _shared:
        _shared.clear() if len(_shared) > 8 else None
        _shared[key] = np.ascontiguousarray(x[b].T).astype(NP_BF16)
    xT = _shared[key]

    heads = [8 * g + i for i in range(4) for _ in (0,)]  # chunk bases
    qcols = []
    for i in range(4):
        h0, h1 = 8 * g + i, 8 * g + i + 4
        qcols.append(wq[:, h0 * D:(h0 + 1) * D])
        qcols.append(wq[:, h1 * D:(h1 + 1) * D])
    kcols = wk[:, 2 * g * D:(2 * g + 2) * D]
    wqkv_c = np.concatenate(qcols + [kcols], axis=1).astype(NP_BF16)
    wv_c = np.ascontiguousarray(wv[:, 2 * g * D:(2 * g + 2) * D]).astype(NP_BF16)
    worows = []
    for i in range(4):
        h0, h1 = 8 * g + i, 8 * g + i + 4
        worows.append(wo[h0 * D:(h0 + 1) * D, :])
        worows.append(wo[h1 * D:(h1 + 1) * D, :])
    wo_c = np.concatenate(worows, axis=0).astype(NP_BF16)

    cosT = np.tile(cos[:S].T, (2, 1)).astype(NP_BF16)
    sinT_h = np.concatenate([-sin[:S].T[:D // 2], sin[:S].T[D // 2:]], axis=0)
    sinT = np.tile(sinT_h, (2, 1)).astype(NP_BF16)
    trimask = np.triu(np.ones((P, P), dtype=NP_BF16))

    return {
        "xT": xT, "wqkv": wqkv_c, "wv": wv_c, "wo": wo_c,
        "cosT": cosT, "sinT": sinT, "trimask": trimask,
    }


def kernel(x, cos, sin, wq, wk, wv, wo):
    x = np.asarray(x)
    S = x.shape[1]
    assert x.shape == (B, S, HID)
    if S not in _CACHE:
        _CACHE[S] = build_nc(S)
    nc = _CACHE[S]
    in_maps = [
        prep_core_inputs(x, np.asarray(cos), np.asarray(sin), np.asarray(wq),
                         np.asarray(wk), np.asarray(wv), np.asarray(wo), core)
        for core in range(8)
    ]
    res = run_bass_kernel_spmd(nc, in_maps, core_ids=list(range(8)))
    out = np.zeros((B, S, HID), np.float32)
    for core in range(8):
        out[core // 4] += res.results[core]["o_part"]
    return out


# revision 31
# speedup vs baseline: 1.3915x; 1.2563x over previous
"""GQA attention (RoPE + causal softmax + o_proj) on 8 Trainium2 NeuronCores.

Sharding: core = b*4 + g where b = batch (2), g = head-group (4).
Each core handles 8 query heads (global 8g..8g+7) and their 2 KV heads
(2g, 2g+1) for one batch element, producing a partial o_proj output
(contraction over its 512 of the 2048 hd dims). The host sums the 4
partials per batch element.

Per-core data layout (all matmul operands bf16, fp32 PSUM accumulation):
  - x arrives pre-transposed (hid, S) so QKV projections contract hid on
    partitions.
  - q^T/k^T are built per 128-row chunk pairing heads (i, i+4) so each
    head's 64 rows sit at the same partition base (0 or 64) as its KV
    head, enabling concurrent 64-row-group score matmuls.
  - Scores are computed transposed: S^T[k, q] = k^T.T @ q^T, softmax
    denominator via a ones column appended to v in the AV matmul
    (PSUM row 64 = sum over k of exp scores).
  - exp (with the 1/sqrt(D) scale folded in) runs on ScalarE straight
    out of PSUM; causal masking multiplies a 128x128 triangular tile
    only on diagonal blocks; fully-masked regions are never computed.
"""

import numpy as np
import ml_dtypes
from contextlib import ExitStack

import concourse.mybir as mybir
from concourse import bacc
from concourse.tile import TileContext
from concourse.bass_utils import run_bass_kernel_spmd

BF16 = mybir.dt.bfloat16
F32 = mybir.dt.float32
NP_BF16 = ml_dtypes.bfloat16

HID = 2048
D = 64
H = 32           # global query heads
KV = 8           # global kv heads
B = 2
P = 128
SC = 512         # q-chunk width (also matmul free dim / PSUM bank)

_CACHE = {}


def build_nc(S):
    assert S % SC == 0
    NHID = HID // P       # hid chunks (16)
    NSB = S // P          # 128-row s-blocks
    NSC = S // SC         # 512-col s-chunks
    QCH = 4               # q chunk-pairs
    EXP = mybir.ActivationFunctionType.Exp

    nc = bacc.Bacc("TRN2", target_bir_lowering=False, debug=False)
    xT = nc.dram_tensor("xT", [HID, S], BF16, kind="ExternalInput")
    wqkv = nc.dram_tensor("wqkv", [HID, 640], BF16, kind="ExternalInput")
    wv = nc.dram_tensor("wv", [HID, 128], BF16, kind="ExternalInput")
    wo = nc.dram_tensor("wo", [512, HID], BF16, kind="ExternalInput")
    cosT = nc.dram_tensor("cosT", [128, S], BF16, kind="ExternalInput")
    sinT = nc.dram_tensor("sinT", [128, S], BF16, kind="ExternalInput")
    trimask = nc.dram_tensor("trimask", [128, 128], BF16, kind="ExternalInput")
    o_part = nc.dram_tensor("o_part", [S, HID], F32, kind="ExternalOutput")

    with TileContext(nc) as tc, ExitStack() as ctx:
        res = ctx.enter_context(tc.tile_pool(name="res", bufs=1))
        rope = ctx.enter_context(tc.tile_pool(name="rope", bufs=2))
        ptp = ctx.enter_context(tc.tile_pool(name="ptp", bufs=8))
        rbp = ctx.enter_context(tc.tile_pool(name="rbp", bufs=2))
        obp = ctx.enter_context(tc.tile_pool(name="obp", bufs=3))
        psum = ctx.enter_context(tc.tile_pool(name="psum", bufs=1, space="PSUM"))

        # resident loads
        cos_sb = res.tile([P, S], BF16, tag="cos")
        nc.sync.dma_start(out=cos_sb, in_=cosT[:, :])
        sin_sb = res.tile([P, S], BF16, tag="sin")
        nc.sync.dma_start(out=sin_sb, in_=sinT[:, :])
        mask_sb = res.tile([P, P], BF16, tag="mask")
        nc.sync.dma_start(out=mask_sb, in_=trimask[:, :])
        # load order matches first-consumption order (proj loop walks h)
        xt_sb, wqkv_sb, wv_sb = [], [], []
        for h in range(NHID):
            t = res.tile([P, S], BF16, tag=f"xt{h}", name=f"xt{h}")
            nc.sync.dma_start(out=t, in_=xT[h * P:(h + 1) * P, :])
            xt_sb.append(t)
            t = res.tile([P, 640], BF16, tag=f"wqkv{h}", name=f"wqkv{h}")
            nc.sync.dma_start(out=t, in_=wqkv[h * P:(h + 1) * P, :])
            wqkv_sb.append(t)
            t = res.tile([P, 128], BF16, tag=f"wv{h}", name=f"wv{h}")
            nc.sync.dma_start(out=t, in_=wv[h * P:(h + 1) * P, :])
            wv_sb.append(t)
        wo_sb = []
        for i in range(4):
            t = res.tile([P, HID], BF16, tag=f"wo{i}", name=f"wo{i}")
            nc.sync.dma_start(out=t, in_=wo[i * P:(i + 1) * P, :])
            wo_sb.append(t)

        # ---- Phase A: q^T/k^T projection + RoPE ----
        # chunks 0-3: q head pairs (i, i+4); chunk 4: k (kv0 rows 0-63, kv1 rows 64-127)
        qkrot = []
        for m in range(5):
            t = res.tile([P, S], BF16, tag=f"qkrot{m}", name=f"qkrot{m}")
            qkrot.append(t)
        # v tiles [128, 194]: [v0(0:64) | 1 | 0 | v1(66:130) | 1 | 0-pad] —
        # both AV stationaries are 128 columns (FWL) at 4B-aligned offsets
        vnat = [res.tile([P, 194], BF16, tag=f"vnat{sb}", name=f"vnat{sb}")
                for sb in range(NSB)]
        attnT = []
        for i in range(QCH):
            t = res.tile([P, S], BF16, tag=f"attnT{i}", name=f"attnT{i}")
            attnT.append(t)

        def emit_o_unit(qb, n):
            po = psum.tile([P, SC], F32, tag="acc", bufs=4, name="po")
            for i in range(QCH):
                nc.tensor.matmul(
                    po,
                    lhsT=attnT[i][:, qb * P:(qb + 1) * P],
                    rhs=wo_sb[i][:, n * SC:(n + 1) * SC],
                    start=(i == 0),
                    stop=(i == QCH - 1),
                )
            ob = obp.tile([P, SC], F32, tag="ob", name="ob")
            nc.vector.tensor_copy(ob, po)
            nc.sync.dma_start(
                out=o_part[qb * P:(qb + 1) * P, n * SC:(n + 1) * SC], in_=ob
            )

        def gen_proj_schunk(s):
            """Emit s-chunk s projections + RoPE + v as units (yields).

            Matmul groups stay consecutive (no spread) and eviction units
            only follow completed groups, so cross-engine ops never
            head-of-line-block the in-order ACT/DVE streams."""
            sl = slice(s * SC, (s + 1) * SC)
            for m in (4, 0, 1, 2, 3):
                ps = psum.tile([P, SC], F32, tag="acc", bufs=4, name="ps_proj")
                for h0 in (0, 8):
                    for h in range(h0, h0 + 8):
                        nc.tensor.matmul(
                            ps,
                            lhsT=wqkv_sb[h][:, m * P:(m + 1) * P],
                            rhs=xt_sb[h][:, sl],
                            start=(h == 0),
                            stop=(h == NHID - 1),
                        )
                    yield
                # rotate_half operand: engines are lane-locked, so the
                # +-32-partition swap must go through DMA (SBUF->SBUF)
                qraw = rope.tile([P, SC], BF16, tag="qraw", bufs=2, name="qraw")
                nc.scalar.copy(qraw, ps)
                qswp = rope.tile([P, SC], BF16, tag="qswp", bufs=2, name="qswp")
                for dst, src in ((0, 32), (32, 0), (64, 96), (96, 64)):
                    nc.sync.dma_start(
                        out=qswp[dst:dst + 32, :], in_=qraw[src:src + 32, :]
                    )
                t1 = rope.tile([P, SC], BF16, tag="t1", bufs=2, name="t1")
                nc.vector.tensor_mul(t1, ps, cos_sb[:, sl])
                t2 = rope.tile([P, SC], BF16, tag="t2", bufs=2, name="t2")
                nc.gpsimd.tensor_mul(t2, qswp, sin_sb[:, sl])
                nc.vector.tensor_add(qkrot[m][:, sl], t1, t2)
                yield
            for sb in range(4 * s, 4 * s + 4):
                t = vnat[sb]
                nc.gpsimd.memset(t[:, 64:65], 1.0)
                nc.gpsimd.memset(t[:, 65:66], 0.0)
                nc.gpsimd.memset(t[:, 130:131], 1.0)
                nc.gpsimd.memset(t[:, 131:194], 0.0)
                pv = psum.tile([P, 128], F32, tag="acc", bufs=4, name="ps_v")
                for h in range(NHID):
                    nc.tensor.matmul(
                        pv,
                        lhsT=xt_sb[h][:, sb * P:(sb + 1) * P],
                        rhs=wv_sb[h],
                        start=(h == 0),
                        stop=(h == NHID - 1),
                    )
                yield
                nc.vector.tensor_copy(t[:, 0:64], pv[:, 0:64])
                nc.vector.tensor_copy(t[:, 66:130], pv[:, 64:128])
                yield

        def gen_o_chunk(c):
            for qb in range(4 * c, 4 * c + 4):
                for n in range(HID // SC):
                    emit_o_unit(qb, n)
                    yield

        def interleave(*gens):
            """Round-robin the generators, one unit per yield."""
            gens = [g for g in gens if g is not None]
            i = 0
            while gens:
                g = gens[i % len(gens)]
                try:
                    next(g)
                except StopIteration:
                    gens.remove(g)
                    continue
                yield
                i += 1

        # ---- all projections emitted dense (interleaving them into the
        # attention loops loses: PE head-of-line blocks on psum-slot waits
        # whose releases depend on cross-engine chains) ----
        for s in range(NSC):
            with nc.named_scope(f"projA{s}"):
                for _ in gen_proj_schunk(s):
                    pass

        # ---- attention chunks; the next s-chunk's projections and the
        # previous chunk's o_proj drip into the kb loop as PE fill work
        # while ScalarE streams exps (the per-kb rate limiter) ----
        for c in range(NSC):
          with nc.named_scope(f"attn_c{c}"):
            q0 = c * SC
            nkb = 4 * c + 4
            filler = interleave(gen_o_chunk(c - 1) if c >= 1 else None)
            n_units = 16 if c >= 1 else 0
            total_iters = 4 * nkb
            it = 0
            spent = 0
            for pg in (0, 1, 2, 3):
                avs = {}
                for hp in (pg,):
                    a0 = psum.tile([P, SC], F32, tag="acc", bufs=4, name="av0")
                    a1 = psum.tile([P, SC], F32, tag="acc", bufs=4, name="av1")
                    avs[hp] = (a0, a1)
                for kb in range(nkb):
                    vs = max(0, (kb - 4 * c) * P)  # first valid col in chunk
                    for hp in (pg,):
                        av0, av1 = avs[hp]
                        st = psum.tile([P, 2 * SC], F32, tag="st", bufs=2, name="st")
                        nc.tensor.matmul(
                            st[:, vs:SC],
                            lhsT=qkrot[4][0:64, kb * P:(kb + 1) * P],
                            rhs=qkrot[hp][0:64, q0 + vs:q0 + SC],
                            start=True, stop=True,
                        )
                        nc.tensor.matmul(
                            st[:, SC:2 * SC],
                            lhsT=qkrot[4][64:128, kb * P:(kb + 1) * P],
                            rhs=qkrot[hp][64:128, q0:q0 + SC],
                            start=True, stop=True,
                        )
                        pt = ptp.tile([P, 2 * SC], BF16, tag="pt", name="pt")
                        # one exp over [vs:1024]: the dead span [SC:SC+vs]
                        # is exp'd too (never read) to save an instruction
                        nc.scalar.activation(
                            pt[:, vs:2 * SC], st[:, vs:2 * SC], EXP, scale=0.125
                        )
                        if kb - 4 * c >= 0:  # diagonal block: mask triangle
                            nc.vector.tensor_mul(
                                pt[:, vs:vs + P], pt[:, vs:vs + P], mask_sb
                            )
                            nc.vector.tensor_mul(
                                pt[:, SC + vs:SC + vs + P],
                                pt[:, SC + vs:SC + vs + P], mask_sb
                            )
                        nc.tensor.matmul(
                            av0[:, vs:SC],
                            lhsT=vnat[kb][:, 0:128],
                            rhs=pt[:, vs:SC],
                            start=(kb == 0), stop=(kb == nkb - 1),
                        )
                        nc.tensor.matmul(
                            av1[:, vs:SC],
                            lhsT=vnat[kb][:, 66:194],
                            rhs=pt[:, SC + vs:2 * SC],
                            start=(kb == 0), stop=(kb == nkb - 1),
                        )
                    it += 1
                    want = (it * n_units) // total_iters
                    while spent < want:
                        try:
                            next(filler)
                            spent += 1
                        except StopIteration:
                            spent = want
                            break
                for hp in (pg,):
                    # normalize: attnT rows = 0 (head hp) / 64 (head hp+4)
                    for av, rbase in ((avs[hp][0], 0), (avs[hp][1], 64)):
                        # compute stays partition-aligned; cross-partition
                        # moves (row 64 -> 0, result -> rows 64..127) via DMA.
                        # reciprocal_approx_fast needs base partition 0.
                        den = rbp.tile([65, SC], F32, tag="den", bufs=2, name="den")
                        nc.vector.tensor_copy(den[64:65, :], av[64:65, :])
                        den0 = rbp.tile([1, SC], F32, tag="den0", bufs=2, name="den0")
                        nc.sync.dma_start(out=den0, in_=den[64:65, :])
                        rec0 = rbp.tile([1, SC], F32, tag="rec0", bufs=2, name="rec0")
                        nc.vector.reciprocal_approx_fast(rec0, den0)
                        rb = rbp.tile([64, SC], F32, tag="rb", bufs=2, name="rb")
                        nc.gpsimd.partition_broadcast(rb, rec0)
                        if rbase == 0:
                            nc.vector.tensor_mul(
                                attnT[hp][0:64, q0:q0 + SC], av[0:64, :], rb
                            )
                        else:
                            hi = rbp.tile([64, SC], BF16, tag="hi", bufs=2, name="hi")
                            nc.vector.tensor_mul(hi, av[0:64, :], rb)
                            nc.sync.dma_start(
                                out=attnT[hp][64:128, q0:q0 + SC], in_=hi
                            )

            # drain remaining filler (next chunk depends on its qkrot/vnat)
            for _ in filler:
                pass
        # last chunk's o_proj tail
        for _ in gen_o_chunk(NSC - 1):
            pass

    nc.finalize()
    return nc


def prep_core_inputs(x, cos, sin, wq, wk, wv, wo, core, _shared={}):
    """Build the per-core input map (all host-side numpy)."""
    b, g = core // 4, core % 4
    S = x.shape[1]

    key = ("xT", b, id(x))
    if key not in _shared:
        _shared.clear() if len(_shared) > 8 else None
        _shared[key] = np.ascontiguousarray(x[b].T).astype(NP_BF16)
    xT = _shared[key]

    heads = [8 * g + i for i in range(4) for _ in (0,)]  # chunk bases
    qcols = []
    for i in range(4):
        h0, h1 = 8 * g + i, 8 * g + i + 4
        qcols.append(wq[:, h0 * D:(h0 + 1) * D])
        qcols.append(wq[:, h1 * D:(h1 + 1) * D])
    kcols = wk[:, 2 * g * D:(2 * g + 2) * D]
    wqkv_c = np.concatenate(qcols + [kcols], axis=1).astype(NP_BF16)
    wv_c = np.ascontiguousarray(wv[:, 2 * g * D:(2 * g + 2) * D]).astype(NP_BF16)
    worows = []
    for i in range(4):
        h0, h1 = 8 * g + i, 8 * g + i + 4
        worows.append(wo[h0 * D:(h0 + 1) * D, :])
        worows.append(wo[h1 * D:(h1 + 1) * D, :])
    wo_c = np.concatenate(worows, axis=0).astype(NP_BF16)

    cosT = np.tile(cos[:S].T, (2, 1)).astype(NP_BF16)
    sinT_h = np.concatenate([-sin[:S].T[:D // 2], sin[:S].T[D // 2:]], axis=0)
    sinT = np.tile(sinT_h, (2, 1)).astype(NP_BF16)
    trimask = np.triu(np.ones((P, P), dtype=NP_BF16))

    return {
        "xT": xT, "wqkv": wqkv_c, "wv": wv_c, "wo": wo_c,
        "cosT": cosT, "sinT": sinT, "trimask": trimask,
    }


def kernel(x, cos, sin, wq, wk, wv, wo):
    x = np.asarray(x)
    S = x.shape[1]
    assert x.shape == (B, S, HID)
    if S not in _CACHE:
        _CACHE[S] = build_nc(S)
    nc = _CACHE[S]
    in_maps = [
        prep_core_inputs(x, np.asarray(cos), np.asarray(sin), np.asarray(wq),
                         np.asarray(wk), np.asarray(wv), np.asarray(wo), core)
        for core in range(8)
    ]
    res = run_bass_kernel_spmd(nc, in_maps, core_ids=list(range(8)))
    out = np.zeros((B, S, HID), np.float32)
    for core in range(8):
        out[core // 4] += res.results[core]["o_part"]
    return out
